# revision 9
# baseline (speedup 1.0000x reference)
"""Per-edge dot product score[e] = h[src[e]] . h[dst[e]] on 8 TRN2 NeuronCores.

v2: hybrid PE-gather (src side) + SWDGE dma_gather (dst side).

The baseline was bound by SWDGE descriptor generation (~8.1 ns/row/queue,
4 queues, 400k gathered rows/core -> ~907 us). This version fetches only
the dst rows via SWDGE (~213k rows/core -> ~430 us) and materializes the
src rows on the Tensor engine via one-hot matmuls:

 - Host bin-packs the 100k nodes into 832 "subs" (<=128 nodes each; 4
   chunks x 208 subs) such that every (sub, dst-chunk) edge run has <=64
   edges. The node->sub assignment is a free host-side permutation of the
   table; each run owns a static 64-column slot, so the whole device
   program is static (no register-offset APs).
 - Per edge group g=(src-chunk A, dst-chunk B), cap 13312 = 208 runs x 64:
   * Pool: baseline-style raw dma_gather of h_pad rows by dst (f32 128B
     payload, int16 chunk-local indices, 2 calls round-robin on 4 queues),
     landing edge-major gd[128, 104, 32] (edge i -> partition i%128).
   * DVE: builds the one-hot (slot x edge) tile by comparing a
     host-replicated uint8 slot stream against a per-partition iota.
   * PE: per run b, matmul(out=gs[64*(b%2):.., ring, :], lhsT=onehot
     [:, 64b:64b+64], rhs=h_sub[128, 32] bf16) -> gs edge-major in PSUM,
     perfectly column-aligned with gd.
   * DVE: mul (PSUM x SBUF) + reduce over D -> score[128, 104] like the
     baseline; host inverse-permutes.
"""

import numpy as np

# problem shape
N_NODES = 100000
D = 32
N_EDGES = 1600000
N_CORES = 8
E_PC = N_EDGES // N_CORES          # 200000

# full-size kernel config
HP = 64                            # h_pad row pitch (f32) -> 256B
RUN = 64                           # columns per (sub, B) run
NCH = 4                            # chunks (both src sub-chunks and dst idx windows)
G = NCH * NCH                      # 16 groups

_CACHE = {}


class _Cfg:
    """Geometry for one compiled kernel (full-size or test-size)."""

    def __init__(self, n_nodes, subs_per_ch, qn, chb=None):
        self.n_nodes = n_nodes
        self.subs_per_ch = subs_per_ch         # subs per src chunk
        self.nsubs = NCH * subs_per_ch
        self.cap = subs_per_ch * RUN           # columns per group
        self.cols = self.cap // 128            # gd free columns
        self.w16 = self.cap // 16              # idx wrap width
        self.qn = qn                           # quanta per group (1024 cols each)
        assert self.qn * 1024 == self.cap
        if chb is None:
            # dst index windows (chunk-local int16); last chunk may be short
            per = subs_per_ch * 128            # nominal nodes per chunk
            chb = [min(i * per, n_nodes) for i in range(NCH + 1)]
        self.chb = list(chb)
        assert self.chb[NCH] == n_nodes
        assert max(self.chb[i + 1] - self.chb[i] for i in range(NCH)) < 32768


CFG_FULL = _Cfg(N_NODES, 208, 13)   # cap 13312, 13 quanta of 1024


def _dma_gather_raw(g, out_ap, in_ap, idxs_ap, num_idxs, num_idxs_reg,
                    elem_size, elem_step, queue_num):
    """bass.dma_gather minus the elem_size%256 assert (the 256B constraint
    is on the DRAM stride; a 128B payload per descriptor is accepted by the
    ucode — same usage as the previous version, verified on HW)."""
    from concourse import ap_utils, mybir
    from concourse.bass import round_up_to_multiple

    g._assert_queue_num(queue_num)
    assert idxs_ap.dtype == mybir.dt.int16
    assert in_ap.dtype == out_ap.dtype
    assert ap_utils.ap_is_contiguous(in_ap.ap[1:])
    assert ap_utils.ap_is_contiguous(out_ap.ap[1:])
    assert ap_utils.ap_is_contiguous(idxs_ap.ap[1:])
    assert num_idxs % 4 == 0
    assert in_ap.ap[-1][1] == elem_size and out_ap.ap[-1][1] == elem_size
    assert out_ap.ap[0][1] * out_ap.ap[1][1] == round_up_to_multiple(num_idxs, 128)
    assert in_ap.ap[0][0] == elem_step
    stride_bytes = elem_step * mybir.dt.size(in_ap.dtype)
    assert stride_bytes % 256 == 0 and stride_bytes // 256 < 256
    _in_ap = g.lower_ap_dma(in_ap, for_custom_bir_dma=True)
    _idxs_ap = g.lower_ap(idxs_ap)
    _out_ap = g.lower_ap(out_ap)
    return g.add_instruction(
        mybir.InstDMAGatherAnt(
            name=g.bass.get_next_instruction_name(),
            ins=[*_in_ap, _idxs_ap, g.lower_val_access(g.to_reg(num_idxs_reg))],
            outs=[_out_ap],
            transpose=False,
            num_idxs=num_idxs,
            elem_size=elem_size,
            stride_bytes_256=stride_bytes // 256,
            gen_mode=0,
            single_packet=False,
            queue_num=queue_num,
            sbuf_tokens_per_rank=0,
            sbuf_free_dim_per_rank=0,
            sbuf_free_dim_pad_per_rank=0,
            sbuf_byte_offset=0,
        )
    )


def _build(cfg):
    from contextlib import ExitStack

    import concourse.bacc as bacc
    import concourse.bass as bass
    from concourse import mybir
    from concourse.library_config import mlp

    CAP, COLS, W16, QN = cfg.cap, cfg.cols, cfg.w16, cfg.qn
    NSUBS, SPC = cfg.nsubs, cfg.subs_per_ch
    NN = cfg.n_nodes
    HALF = CAP // 2                       # per gather call
    RPQ = 1024 // RUN                     # runs per quantum = 16
    PPQ = 8                               # pairs per quantum (128-edge pairs)

    nc = bacc.Bacc("TRN2", target_bir_lowering=False, debug=False,
                   num_swdge_queues=4)

    h_pad = nc.dram_tensor("h_pad", [NN, HP], mybir.dt.float32,
                           kind="ExternalInput")
    h_sb_d = nc.dram_tensor("h_sb", [128, NSUBS, D], mybir.dt.bfloat16,
                            kind="ExternalInput")
    idx_d = nc.dram_tensor("idx", [G, 128, W16], mybir.dt.int16,
                           kind="ExternalInput")
    slr_d = nc.dram_tensor("slr", [G, 16, CAP], mybir.dt.uint8,
                           kind="ExternalInput")
    iota_d = nc.dram_tensor("iota", [128, 1], mybir.dt.float32,
                            kind="ExternalInput")
    score = nc.dram_tensor("score", [G, 128, COLS], mybir.dt.float32,
                           kind="ExternalOutput")

    def chunk_ap(b):
        return h_pad[cfg.chb[b]:cfg.chb[b + 1], :D]

    with (
        nc.Block() as block,
        nc.sbuf_tensor("hsb", [128, NSUBS, D], mybir.dt.bfloat16) as hsb,
        nc.sbuf_tensor("iot", [128, 1], mybir.dt.float32) as iot,
        nc.sbuf_tensor("ix", [128, 2, W16], mybir.dt.int16) as ix,
        nc.sbuf_tensor("gd", [128, 2, COLS, D], mybir.dt.float32) as gd,
        nc.sbuf_tensor("slb", [128, 2, CAP], mybir.dt.uint8) as slb,
        nc.sbuf_tensor("oh", [128, 2, 1024], mybir.dt.bfloat16) as oh,
        nc.sbuf_tensor("sc", [128, 2, COLS], mybir.dt.float32) as sc,
        nc.psum_tensor("gs", [128, 16, D], mybir.dt.float32) as gs,
        nc.semaphore("s_hsb") as s_hsb,
        nc.semaphore("s_oh") as s_oh,       # onehot quanta built
        nc.semaphore("s_mm") as s_mm,       # PE quanta done
        nc.semaphore("s_red") as s_red,     # DVE mul+reduce quanta done
        nc.semaphore("s_mr") as s_mr,       # DVE mul->reduce pipeline sync
        ExitStack() as stack,
    ):
        qs = [stack.enter_context(nc.semaphore(f"q{i}")) for i in range(4)]  # noqa: ANT232
        s_ixd = [stack.enter_context(nc.semaphore(f"s_ixd{i}")) for i in range(2)]  # noqa: ANT232
        s_slb = [stack.enter_context(nc.semaphore(f"s_slb{i}")) for i in range(2)]  # noqa: ANT232
        s_out = [stack.enter_context(nc.semaphore(f"s_out{i}")) for i in range(2)]  # noqa: ANT232

        @block.sync
        def _(sp):
            sp.dma_start(hsb[:], h_sb_d[:]).then_inc(s_hsb, 16)
            sp.dma_start(iot[:], iota_d[:]).then_inc(s_hsb, 16)
            for g in range(G + 2):
                if g < G:
                    s = g % 2
                    if g >= 2:
                        # ix/slb buffer s reusable once group g-2's desc-gen
                        # (implied by its gather DMA sems) and onehot builds
                        # consumed them
                        sp.wait_ge(qs[(2 * (g - 2)) % 4],
                                   16 * ((2 * (g - 2)) // 4 + 1))
                        sp.wait_ge(qs[(2 * (g - 2) + 1) % 4],
                                   16 * ((2 * (g - 2) + 1) // 4 + 1))
                        sp.wait_ge(s_oh, QN * (g - 1))
                    sp.dma_start(ix[:, s], idx_d[g]).then_inc(s_ixd[s], 16)
                    for k in range(8):
                        sp.dma_start(slb[16 * k:16 * (k + 1), s],
                                     slr_d[g]).then_inc(s_slb[s], 16)
                if g >= 2:
                    sp.wait_ge(s_red, QN * (g - 1))
                    sp.dma_start(score[g - 2],
                                 sc[:, g % 2]).then_inc(s_out[g % 2], 16)

        @block.gpsimd
        def _(gp):
            gp.load_library(mlp)
            for g in range(G):
                s = g % 2
                b = g % NCH
                gp.wait_ge(s_ixd[s], 16 * (g // 2 + 1))      # ix of group g
                if g >= 2:
                    gp.wait_ge(s_red, QN * (g - 1))          # gd buf s free
                for c in range(2):
                    k = 2 * g + c
                    if k >= 4:
                        gp.wait_ge(qs[k % 4], 16 * (k // 4))
                    _dma_gather_raw(
                        gp,
                        gd[:, s, c * (COLS // 2):(c + 1) * (COLS // 2)],
                        chunk_ap(b),
                        ix[:, s, c * (W16 // 2):(c + 1) * (W16 // 2)],
                        HALF, HALF,
                        D, HP, queue_num=k % 4,
                    ).then_inc(qs[k % 4], 16)

        @block.vector
        def _(v):
            from concourse import mybir as mb
            v.wait_ge(s_hsb, 32)
            for g in range(G):
                s = g % 2
                for q in range(QN):
                    t = QN * g + q
                    # build onehot quantum: oh[p, i] = (slb[p, i] == p)
                    v.wait_ge(s_slb[s], 128 * (g // 2 + 1))
                    if t >= 2:
                        v.wait_ge(s_mm, t - 1)       # oh buf reusable
                    v.tensor_scalar(
                        oh[:, t % 2], slb[:, s, 1024 * q:1024 * (q + 1)],
                        iot[:], None, op0=mb.AluOpType.is_equal,
                    ).then_inc(s_oh, 1)
                    # mul+reduce of quantum t (PE result + gd needed)
                    v.wait_ge(s_mm, t + 1)
                    if q == 0 and g >= 2:
                        v.wait_ge(s_out[s], 16 * (g // 2))   # sc buf s free
                    # quantum q covers gd cols [8q, 8q+8); gather call 0
                    # holds cols [0, COLS/2), call 1 the rest
                    c0 = 1 if (8 * q + 8) > (COLS // 2) else 0
                    cq = (2 * g + c0) % 4
                    v.wait_ge(qs[cq], 16 * ((2 * g + c0) // 4 + 1))
                    if c0 == 1 and 8 * q < (COLS // 2):
                        cq0 = (2 * g) % 4
                        v.wait_ge(qs[cq0], 16 * ((2 * g) // 4 + 1))
                    v.tensor_mul(
                        gs[:, 8 * (q % 2):8 * (q % 2) + 8, :],
                        gs[:, 8 * (q % 2):8 * (q % 2) + 8, :],
                        gd[:, s, 8 * q:8 * q + 8, :],
                    ).then_inc(s_mr, 1)
                    v.wait_ge(s_mr, t + 1)
                    v.tensor_reduce(
                        sc[:, s, 8 * q:8 * q + 8],
                        gs[:, 8 * (q % 2):8 * (q % 2) + 8, :],
                        axis=mb.AxisListType.X, op=mb.AluOpType.add,
                    ).then_inc(s_red, 1)

        @block.tensor
        def _(pe):
            pe.wait_ge(s_hsb, 32)
            for g in range(G):
                a = g // NCH
                for q in range(QN):
                    t = QN * g + q
                    pe.wait_ge(s_oh, t + 1)
                    if t >= 2:
                        pe.wait_ge(s_red, t - 1)     # psum ring half free
                    mm = None
                    for r in range(RPQ):             # 16 runs of 64 cols
                        blk = RPQ * q + r            # run index within group
                        sub = SPC * a + blk
                        pair = (8 * q + r // 2) % 16  # psum ring slot
                        mm = pe.matmul(
                            gs[64 * (r % 2):64 * (r % 2) + 64,
                               pair:pair + 1, :],
                            oh[:, t % 2, 64 * r:64 * (r + 1)],
                            hsb[:, sub, :],
                            start=True, stop=True,
                        )
                    mm.then_inc(s_mm, 1)

    nc.compile()
    return nc


def _get_nc(cfg):
    key = ("nc", cfg.cap, tuple(cfg.chb))
    if key not in _CACHE:
        _CACHE[key] = _build(cfg)
    return _CACHE[key]


def _pack_nodes(m, cfg, rng_shift=0):
    """Assign nodes to subs: <=128 nodes/sub, per-B load <= RUN.

    m: [n_nodes, 4] edge multiplicities (this core). Batched LPT: process
    nodes heaviest-first in batches of nsubs, pairing the heaviest nodes
    with the least-loaded subs; then repair any (sub, B) overflow by
    moving contributing nodes to slack subs."""
    NS = cfg.nsubs
    n = m.shape[0]
    # stripe nodes across subs in m-vector class order: every class lands
    # near-evenly on every sub, so per-(sub, B) loads start within a few
    # units of the mean
    key = (np.minimum(m, 63) * (64 ** np.arange(4))).sum(1)
    order = np.argsort(key, kind="stable")
    if rng_shift:
        order = np.roll(order, rng_shift)
    sub_of = np.empty(n, dtype=np.int32)
    sub_of[order] = np.arange(n) % NS
    loads = np.zeros((NS, 4), dtype=np.int64)
    np.add.at(loads, sub_of, m)
    counts = np.bincount(sub_of, minlength=NS).astype(np.int64)

    # repair (sub, B) overflows: move the smallest sufficient contributor to
    # the emptiest sub that can take it
    nodes_by_sub = [list(np.where(sub_of == s)[0]) for s in range(NS)]
    for _ in range(50000):
        over_amt = loads - RUN
        worst = np.unravel_index(np.argmax(over_amt), over_amt.shape)
        if over_amt[worst] <= 0:
            break
        s, b = int(worst[0]), int(worst[1])
        cand_nodes = [v for v in nodes_by_sub[s] if m[v, b] > 0]
        cand_nodes.sort(key=lambda v: m[v, b])
        moved = False
        for v in cand_nodes:
            ok = ((loads + m[v][None, :]) <= RUN).all(1) & (counts < 128)
            ok[s] = False
            tgt_c = np.where(ok)[0]
            if len(tgt_c) == 0:
                continue
            tgt = int(tgt_c[np.argmin(loads[tgt_c].sum(1) * 4 + counts[tgt_c])])
            loads[s] -= m[v]
            counts[s] -= 1
            loads[tgt] += m[v]
            counts[tgt] += 1
            sub_of[v] = tgt
            nodes_by_sub[s].remove(v)
            nodes_by_sub[tgt].append(v)
            moved = True
            break
        if not moved:
            # swap: exchange a contributor with a zero-m_b node elsewhere
            done = False
            for v in cand_nodes:
                dm = m[v]
                for tgt in np.argsort(loads[:, b]):
                    if tgt == s:
                        continue
                    for u in nodes_by_sub[int(tgt)]:
                        if m[u, b] >= dm[b]:
                            continue
                        nl_t = loads[tgt] + dm - m[u]
                        nl_s = loads[s] - dm + m[u]
                        if (nl_t <= RUN).all() and (nl_s[b] < loads[s, b]):
                            loads[tgt] = nl_t
                            loads[s] = nl_s
                            sub_of[v] = tgt
                            sub_of[u] = s
                            nodes_by_sub[s].remove(v)
                            nodes_by_sub[tgt].remove(u)
                            nodes_by_sub[s].append(u)
                            nodes_by_sub[int(tgt)].append(v)
                            done = True
                            break
                    if done:
                        break
                if done:
                    break
            if not done:
                raise _PackFail()
    else:
        raise _PackFail()
    if (counts > 128).any() or (loads > RUN).any():
        raise _PackFail()
    return sub_of, counts


class _PackFail(Exception):
    pass


def _prep_core(h_pad, h, src, dst, cfg):
    """Host marshaling for one core. Returns (in_map, perm_pos, valid)."""
    CAP, SPC, W16 = cfg.cap, cfg.subs_per_ch, cfg.w16
    NN = cfg.n_nodes
    chb = np.asarray(cfg.chb)

    B = np.searchsorted(chb, dst, side="right") - 1
    m = np.zeros((NN, 4), dtype=np.int64)
    np.add.at(m, (src, B), 1)

    sub_of = None
    for shift in (0, 1237, 77777):
        try:
            sub_of, counts = _pack_nodes(m, cfg, rng_shift=shift)
            break
        except _PackFail:
            continue
    if sub_of is None:
        raise RuntimeError("node packing failed")

    # slots within subs
    order_n = np.argsort(sub_of, kind="stable")
    starts = np.zeros(cfg.nsubs + 1, dtype=np.int64)
    np.cumsum(np.bincount(sub_of, minlength=cfg.nsubs), out=starts[1:])
    slot_of = np.empty(NN, dtype=np.int64)
    slot_of[order_n] = np.arange(NN) - np.repeat(starts[:-1], np.diff(starts))

    # h_sb table: [128, nsubs, 32] bf16
    try:
        import ml_dtypes
        bf16 = ml_dtypes.bfloat16
    except ImportError:
        bf16 = np.float32  # fallback; bass casts on upload
    h_sb = np.zeros((128, cfg.nsubs, D), dtype=np.float32)
    h_sb[slot_of, sub_of] = h
    h_sb = h_sb.astype(bf16)

    # edge placement
    sub_e = sub_of[src]
    a_e = sub_e // SPC
    g_e = a_e * NCH + B
    run_e = sub_e % SPC
    key = g_e * SPC + run_e
    order_e = np.argsort(key, kind="stable")
    ks = key[order_e]
    run_starts = np.searchsorted(ks, np.arange(G * SPC))
    run_counts = np.diff(np.concatenate([run_starts, [len(ks)]]))
    if run_counts.max() > RUN:
        raise RuntimeError(f"run overflow {run_counts.max()}")
    within = np.arange(len(ks)) - np.repeat(run_starts, run_counts)
    col = (ks % SPC) * RUN + within          # position within group
    ge = ks // SPC

    dst_local = (dst - chb[B]).astype(np.int16)

    idx = np.zeros((G, CAP), dtype=np.int16)
    slr = np.full((G, CAP), 255, dtype=np.uint8)
    idx[ge, col] = dst_local[order_e]
    slr[ge, col] = slot_of[src[order_e]].astype(np.uint8)

    # wrap idx to ucode layout [16, CAP/16] replicated to 128 partitions
    w = idx.reshape(G, W16, 16).transpose(0, 2, 1)          # [G, 16, W16]
    idx_wrap = np.ascontiguousarray(
        np.broadcast_to(w[:, None], (G, 8, 16, W16)).reshape(G, 128, W16))

    slr16 = np.ascontiguousarray(
        np.broadcast_to(slr[:, None, :], (G, 16, CAP)))

    iota = np.arange(128, dtype=np.float32)[:, None]

    in_map = {
        "h_pad": h_pad,
        "h_sb": h_sb,
        "idx": idx_wrap,
        "slr": slr16,
        "iota": iota,
    }
    perm_pos = np.empty(len(src), dtype=np.int64)
    perm_pos[order_e] = ge * CAP + col       # flat score position per edge
    return in_map, perm_pos


def run(h, src, dst, trace=False, cfg=CFG_FULL):
    """Returns (score [N_EDGES, 1] float32, exec_time_ns or None)."""
    from concourse.bass_utils import run_bass_kernel_spmd

    h = np.asarray(h, dtype=np.float32)
    src = np.asarray(src).astype(np.int64)
    dst = np.asarray(dst).astype(np.int64)
    n_edges = len(src)
    epc = n_edges // N_CORES

    # balance dst chunks against the input's dst distribution (the harness
    # RNG is visibly non-uniform); boundaries are global, so one module
    # serves all cores
    hist = np.bincount(dst, minlength=cfg.n_nodes)
    cum = np.cumsum(hist)
    chb = [0]
    for i in range(1, NCH):
        chb.append(int(np.searchsorted(cum, cum[-1] * i / NCH)))
    chb.append(cfg.n_nodes)
    if max(chb[i + 1] - chb[i] for i in range(NCH)) < 32768:
        cfg = _Cfg(cfg.n_nodes, cfg.subs_per_ch, cfg.qn, chb=chb)

    h_pad = np.zeros((cfg.n_nodes, HP), dtype=np.float32)
    h_pad[:, :D] = h

    in_maps, perms = [], []
    for c in range(N_CORES):
        sl = slice(c * epc, (c + 1) * epc)
        im, pp = _prep_core(h_pad, h, src[sl], dst[sl], cfg)
        in_maps.append(im)
        perms.append(pp)

    nc = _get_nc(cfg)
    res = run_bass_kernel_spmd(nc, in_maps, list(range(N_CORES)), trace=trace)
    _CACHE["last_res"] = res

    out = np.empty(n_edges, dtype=np.float32)
    for c in range(N_CORES):
        scr = res.results[c]["score"]                 # [G, 128, COLS]
        flat = scr.transpose(0, 2, 1).reshape(-1)     # pos = g*CAP + col*...
        out[c * epc:(c + 1) * epc] = flat[perms[c]]
    return out.reshape(n_edges, 1), res.exec_time_ns


def kernel(h, src, dst):
    out, _ = run(h, src, dst, trace=False)
    return out
